# revision 2
# baseline (speedup 1.0000x reference)
"""Batched MoE (top-2, 8 experts) on 8 Trainium2 NeuronCores.

Strategy: d_ff-slice parallel (tensor-parallel on the hidden dim). Every
core processes ALL experts and ALL routed token-columns, but only a
512-wide slice of d_ff: core s owns w1[:, :, 512s:512(s+1)],
w2[:, :, 512s:512(s+1)], w3[:, 512s:512(s+1), :]. Each core produces a
partial output (its slice's contribution to hidden @ w3); the 8 partials
are summed on the host, which also does routing (sort by expert) and the
top-k combine — cheap O(tokens·d) index ops; all matmul FLOPs run on
device.

Why this beats expert-parallel: expert-parallel pays the MAX per-expert
token count on every core (SPMD capacity), ~10% above the mean for this
routing. The f-slice split gives every core exactly Sum(counts) columns
— the FLOP lower bound (12.06 GFLOP/core) — for ANY routing, and the
program is identical across cores (only the weight *content* differs per
core), so SPMD is exact. Tokens whose two top-k slots picked the SAME
expert are merged into one assignment with combine weight (w0+w1).

Device dataflow per core (per expert e with Ce columns, f' = local slice
coordinate 0..511):
    xt_e = X_e^T                [1024, Ce]   (d on partitions, k-tiled)
    GT   = w1s[e]^T @ xt_e      [512, Ce]    8 k-matmuls per 128-row f-tile
    VT   = w2s[e]^T @ xt_e      [512, Ce]
    HT   = silu(GT) * VT        [512, Ce]    bf16
    OT_s = w3s[e]^T @ HT        [1024, Ce]   4 jj-matmuls accumulate in PSUM
OT_s goes out in bf16; host sums the 8 slices in fp32.

All matmul operands bf16, PSUM fp32. The layer-3 contraction per core is
only 512 (= 4 PSUM-accumulated matmuls), so no SBUF fp32 accumulator is
needed at all (vs the expert-parallel version's 8-chunk accumulation).

DMA shaping: weights/xt/ot are host-packed into [128, *] buffers whose
partition lines are contiguous in DRAM (7.5-16 KB lines). Expert 0's
weights are loaded as 16 per-k-slab DMAs ([128,512] each) so the first
matmul only waits on ~250 KB; everything later moves in fat per-expert
DMAs in PE consumption order, gated behind stage 1 (a tiny DVE reader
serializes the SP-ring FIFO) so startup bandwidth goes to the critical
bytes. Phase B (OT for expert e) is issued after phase A of expert e+1
so the PE never waits on the ACT/DVE epilogue that produces HT. OT
stores go out per m-pair, interleaved with the matmuls.
"""

import numpy as np

N_EXPERTS = 8
N_CORES = 8
D_MODEL = 1024
D_FF = 4096
FSLICE = D_FF // N_CORES   # 512 f-columns per core
JT = FSLICE // 128         # 4 f-tiles per core's slice
KT = D_MODEL // 128        # 8 k-tiles (contraction d)
MT = D_MODEL // 128        # 8 output d-tiles

_program_cache = {}


def _col_chunks(C):
    """Split C columns into <=512 even pieces (PSUM bank limit, fp32)."""
    if C <= 512:
        return [(0, C)]
    n = (C + 511) // 512
    base = C // n
    rem = C - base * n
    out = []
    off = 0
    for i in range(n):
        sz = base + (1 if i < rem else 0)
        out.append((off, sz))
        off += sz
    return out


def _build_program(caps):
    import concourse.bacc as bacc
    import concourse.mybir as mybir
    from concourse.tile import TileContext

    BF16 = mybir.dt.bfloat16
    F32 = mybir.dt.float32
    SILU = mybir.ActivationFunctionType.Silu

    caps = list(caps)
    E = len(caps)
    offs = [sum(caps[:i]) for i in range(E)]   # column offset of expert block
    AT = sum(caps)
    CMAX = max(caps)

    nc = bacc.Bacc()
    # xt: per-expert blocks, k-major inside a block: [128, sum_e 8k*Ce]
    xt_d = nc.declare_dram_parameter("xt", [128, KT * AT], BF16, isOutput=False)
    # w12: per-expert: w1 slice [8k x 512f'] then w2 slice: [128, 8e*2*4096]
    w12_d = nc.declare_dram_parameter(
        "w12", [128, E * 2 * KT * FSLICE], BF16, isOutput=False
    )
    # w3: per-expert: [4j x 1024m]: [128, 8e*4096]
    w3_d = nc.declare_dram_parameter(
        "w3p", [128, E * JT * D_MODEL], BF16, isOutput=False
    )
    # ot: per-expert blocks, m-major inside: [128, sum_e 8m*Ce]
    ot_d = nc.declare_dram_parameter("ot", [128, MT * AT], BF16, isOutput=True)

    with TileContext(nc) as tc:
        with (
            tc.tile_pool(name="xtp", bufs=1) as xt_pool,
            tc.tile_pool(name="w12", bufs=3) as w12_pool,
            tc.tile_pool(name="w3p", bufs=2) as w3_pool,
            tc.tile_pool(name="htp", bufs=2) as ht_pool,
            tc.tile_pool(name="ot16", bufs=2) as ot16_pool,
            tc.tile_pool(name="tmp", bufs=4) as tmp_pool,
            tc.tile_pool(name="pg", bufs=2, space="PSUM") as pg_pool,
            tc.tile_pool(name="pv", bufs=2, space="PSUM") as pv_pool,
            tc.tile_pool(name="po", bufs=3, space="PSUM") as po_pool,
            tc.tile_pool(name="pw", bufs=1, space="PSUM") as pw_pool,
        ):
            xt_sb = xt_pool.tile([128, KT * AT], BF16, tag="xt", name="xt")

            def xte(e, k, c0, cl):
                base = KT * offs[e] + k * caps[e]
                return xt_sb[:, base + c0 : base + c0 + cl]

            def load_xt(e, npieces):
                base = KT * offs[e]
                w = KT * caps[e]
                step = -(-KT // npieces)  # k-tiles per piece
                for k0 in range(0, KT, step):
                    k1 = min(k0 + step, KT)
                    nc.sync.dma_start(
                        out=xt_sb[:, base + k0 * caps[e] : base + k1 * caps[e]],
                        in_=xt_d[:, base + k0 * caps[e] : base + k1 * caps[e]],
                    )

            # w12 SBUF tile per expert: [128, 2*8k*512f'] laid out as DRAM
            def load_w12(e):
                off = e * 2 * KT * FSLICE
                t = w12_pool.tile(
                    [128, 2 * KT * FSLICE], BF16, tag="w12", name=f"w12e{e}"
                )
                h = KT * FSLICE
                nc.sync.dma_start(out=t[:, :h], in_=w12_d[:, off : off + h])
                nc.sync.dma_start(out=t[:, h:], in_=w12_d[:, off + h : off + 2 * h])
                return t

            def load_stage1():
                """Expert 0's xt + w1/w2 in PE consumption order (w12 is
                jj-major): first matmul waits on ~400KB, and each jj group
                only needs ~0.5MB more — under the early DMA ramp rate."""
                t = w12_pool.tile(
                    [128, 2 * KT * FSLICE], BF16, tag="w12", name="w12e0"
                )
                c0 = caps[0]
                JB = KT * 128  # one jj block: 8k x 128f cols

                def jblk(half, jj):
                    s = half * KT * FSLICE + jj * JB
                    nc.sync.dma_start(
                        out=t[:, s : s + JB], in_=w12_d[:, s : s + JB]
                    )

                jblk(0, 0)
                nc.sync.dma_start(out=xt_sb[:, :c0], in_=xt_d[:, :c0])
                jblk(1, 0)
                for k in range(1, KT):
                    nc.sync.dma_start(
                        out=xt_sb[:, k * c0 : (k + 1) * c0],
                        in_=xt_d[:, k * c0 : (k + 1) * c0],
                    )
                for jj in range(1, JT):
                    jblk(0, jj)
                    jblk(1, jj)
                return t

            def load_w3(e):
                off = e * JT * D_MODEL
                t = w3_pool.tile(
                    [128, JT * D_MODEL], BF16, tag="w3", name=f"w3e{e}"
                )
                nc.sync.dma_start(out=t[:], in_=w3_d[:, off : off + JT * D_MODEL])
                return t

            def phase_a(e, w12):
                """GT/VT matmuls + silu*mul epilogue -> HT tiles for expert e."""
                ccs = _col_chunks(caps[e])
                hts = []
                for jj in range(JT):
                    ht_t = ht_pool.tile(
                        [128, CMAX], BF16, tag=f"ht{jj}", name=f"ht{jj}"
                    )
                    for c0, cl in ccs:
                        pg = pg_pool.tile([128, 512], F32, tag="pg", name="pg")
                        pv = pv_pool.tile([128, 512], F32, tag="pv", name="pv")
                        for k in range(KT):
                            # w12 is jj-major: slab (jj, k) at jj*8k*128 + k*128
                            ws = slice(
                                jj * KT * 128 + k * 128, jj * KT * 128 + (k + 1) * 128
                            )
                            nc.tensor.matmul(
                                out=pg[:, :cl],
                                lhsT=w12[:, ws],
                                rhs=xte(e, k, c0, cl),
                                start=(k == 0),
                                stop=(k == KT - 1),
                            )
                        for k in range(KT):
                            ws = slice(
                                KT * FSLICE + jj * KT * 128 + k * 128,
                                KT * FSLICE + jj * KT * 128 + (k + 1) * 128,
                            )
                            nc.tensor.matmul(
                                out=pv[:, :cl],
                                lhsT=w12[:, ws],
                                rhs=xte(e, k, c0, cl),
                                start=(k == 0),
                                stop=(k == KT - 1),
                            )
                        st = tmp_pool.tile([128, 512], F32, tag="silu", name="st")
                        nc.scalar.activation(st[:, :cl], pg[:, :cl], SILU)
                        nc.vector.tensor_mul(
                            out=ht_t[:, c0 : c0 + cl], in0=st[:, :cl], in1=pv[:, :cl]
                        )
                    hts.append(ht_t)
                return hts

            def phase_b(e, w3c, hts):
                """OT = w3s^T @ HT for expert e; stores per m-pair."""
                ce = caps[e]
                ccs = _col_chunks(ce)
                ot_t = ot16_pool.tile(
                    [128, MT * CMAX], BF16, tag="ot16", name=f"ot16e{e}"
                )
                for m in range(MT):
                    for c0, cl in ccs:
                        po = po_pool.tile([128, 512], F32, tag="po", name="po")
                        for jj in range(JT):
                            ws = slice(jj * D_MODEL + m * 128, jj * D_MODEL + (m + 1) * 128)
                            nc.tensor.matmul(
                                out=po[:, :cl],
                                lhsT=w3c[:, ws],
                                rhs=hts[jj][:, c0 : c0 + cl],
                                start=(jj == 0),
                                stop=(jj == JT - 1),
                            )
                        if e == E - 1 and m == MT - 1:
                            # final copy split across Vector+Scalar so the
                            # tail copy takes half the time
                            h = (cl // 2) // 2 * 2
                            nc.vector.tensor_copy(
                                out=ot_t[:, m * ce + c0 : m * ce + c0 + h],
                                in_=po[:, :h],
                            )
                            nc.scalar.activation(
                                ot_t[:, m * ce + c0 + h : m * ce + c0 + cl],
                                po[:, h:cl],
                                mybir.ActivationFunctionType.Copy,
                            )
                        else:
                            nc.vector.tensor_copy(
                                out=ot_t[:, m * ce + c0 : m * ce + c0 + cl],
                                in_=po[:, :cl],
                            )
                    if m % 2 == 1:
                        base = MT * offs[e] + (m - 1) * ce
                        if e == E - 1 and m == MT - 1:
                            # last pair: two half-stores so the final drain
                            # overlaps the closing matmuls
                            nc.sync.dma_start(
                                out=ot_d[:, base : base + ce],
                                in_=ot_t[:, (m - 1) * ce : m * ce],
                            )
                            nc.sync.dma_start(
                                out=ot_d[:, base + ce : base + 2 * ce],
                                in_=ot_t[:, m * ce : (m + 1) * ce],
                            )
                        else:
                            nc.sync.dma_start(
                                out=ot_d[:, base : base + 2 * ce],
                                in_=ot_t[:, (m - 1) * ce : (m + 1) * ce],
                            )

            # ---- schedule ----
            # PE pre-warm: ~10 matmuls on a memset tile, no DMA deps, so
            # they run during the ~10us DMA/queue spin-up and flip HAM to
            # K=8/8 (2.4 GHz) before the first real matmul issues.
            wt = tmp_pool.tile([128, 128], BF16, tag="warm", name="warm")
            nc.vector.memset(wt[:], 0.0)
            pw = pw_pool.tile([128, 512], F32, tag="pw", name="pw")
            for _ in range(29):
                nc.tensor.matmul(
                    out=pw[:, :128],
                    lhsT=wt[:],
                    rhs=wt[:],
                    start=True,
                    stop=True,
                )
            # stage 1: only what the first matmul groups need: expert 0's
            # xt k-slabs + w1/w2 k-slabs, finely interleaved so the first
            # real matmul only waits on ~250KB.
            w12 = load_stage1()
            # per-phase loads in PE consumption order. Loads for phase e+1
            # are emitted BEFORE phase B(e-1) (whose stores would otherwise
            # sit ahead of them in the sync-ring FIFO and delay them until
            # B(e-1)'s data is ready).
            load_xt(1, 1)
            w12_nxt = load_w12(1)
            hts_prev = phase_a(0, w12)
            for e in range(1, E):
                w3_prev = load_w3(e - 1)
                w12_nn = None
                if e + 1 < E:
                    load_xt(e + 1, 1)
                    w12_nn = load_w12(e + 1)
                hts = phase_a(e, w12_nxt)
                phase_b(e - 1, w3_prev, hts_prev)
                hts_prev = hts
                w12_nxt = w12_nn
            w3_last = load_w3(E - 1)
            phase_b(E - 1, w3_last, hts_prev)

    nc.compile()
    return nc


def _get_program(caps):
    key = tuple(caps)
    if key not in _program_cache:
        _program_cache[key] = _build_program(caps)
    return _program_cache[key]


def _run(nc, in_maps, trace=False):
    import time

    from concourse.bass_utils import run_bass_kernel_spmd

    last = None
    for attempt in range(4):
        try:
            return run_bass_kernel_spmd(
                nc, in_maps, list(range(N_CORES)), trace=trace
            )
        except Exception as e:  # stale device state from a prior crashed run
            last = e
            time.sleep(10 * (attempt + 1))
            try:  # poke the runtime with a trivial op to clear/verify state
                import jax
                import jax.numpy as jnp

                jnp.add(jnp.ones((8, 8)), 1.0).block_until_ready()
            except Exception:
                pass
    raise last


def kernel(x, expert_indices, expert_weights, w1, w2, w3, _trace=False):
    import ml_dtypes

    BF16 = ml_dtypes.bfloat16

    x = np.ascontiguousarray(np.asarray(x, dtype=np.float32))
    expert_indices = np.asarray(expert_indices)
    expert_weights = np.asarray(expert_weights, dtype=np.float32)
    w1 = np.asarray(w1, dtype=np.float32)
    w2 = np.asarray(w2, dtype=np.float32)
    w3 = np.asarray(w3, dtype=np.float32)

    n_tokens, d_model = x.shape
    n_experts = w1.shape[0]

    # assignments with [e,e] top-2 duplicates merged (weight w0+w1) — exact
    e0, e1 = (
        expert_indices[:, 0].astype(np.int64),
        expert_indices[:, 1].astype(np.int64),
    )
    wt0, wt1 = expert_weights[:, 0], expert_weights[:, 1]
    dup = e0 == e1
    a_tok = np.concatenate([np.arange(n_tokens), np.arange(n_tokens)[~dup]])
    a_e = np.concatenate([e0, e1[~dup]])
    a_w = np.concatenate([np.where(dup, wt0 + wt1, wt0), wt1[~dup]]).astype(
        np.float32
    )

    order = np.argsort(a_e, kind="stable")
    s_tok = a_tok[order]
    s_w = a_w[order]
    counts = np.bincount(a_e, minlength=n_experts)
    starts = np.concatenate([[0], np.cumsum(counts)[:-1]])

    # process experts in descending-size order: the last (smallest) expert
    # minimizes the tail (final copies + stores after the last matmul)
    porder = np.argsort(-counts, kind="stable")
    caps = [max(4, -(-int(counts[e]) // 2) * 2) for e in porder]  # 2-col align
    offs = [sum(caps[:i]) for i in range(n_experts)]
    AT = sum(caps)

    x16 = x.astype(BF16)

    # xt: [128, KT*AT] — per-block (= processed expert), k-major lines
    xt = np.zeros((128, KT * AT), BF16)
    for i, e in enumerate(porder):
        seg = s_tok[starts[e] : starts[e] + counts[e]]
        blk = np.zeros((KT, 128, caps[i]), BF16)
        blk[:, :, : counts[e]] = x16[seg].T.reshape(KT, 128, counts[e])
        xt[:, KT * offs[i] : KT * (offs[i] + caps[i])] = blk.transpose(
            1, 0, 2
        ).reshape(128, KT * caps[i])
    xt = np.ascontiguousarray(xt)

    # per-core weight slices
    # w1 [e, 1024, 4096] -> [8s][e][128p][8k*512f']
    w1v = w1.astype(BF16).reshape(n_experts, KT, 128, N_CORES, FSLICE)
    w1t = w1v.transpose(3, 0, 2, 1, 4)  # [s, e, p, k, f']
    w2v = w2.astype(BF16).reshape(n_experts, KT, 128, N_CORES, FSLICE)
    w2t = w2v.transpose(3, 0, 2, 1, 4)
    # w3 [e, 4096, 1024] -> [8s][e][128p][4j*1024m]
    w3v = w3.astype(BF16).reshape(n_experts, N_CORES, JT, 128, D_MODEL)
    w3t = w3v.transpose(1, 0, 3, 2, 4)  # [s, e, p, j, m]

    in_maps = []
    for s in range(N_CORES):
        w12c = np.empty((128, n_experts * 2 * KT * FSLICE), BF16)
        for i, e in enumerate(porder):
            off = i * 2 * KT * FSLICE
            # jj-major within each half: [p][jj][k][128f]
            w12c[:, off : off + KT * FSLICE] = (
                w1t[s, e]
                .reshape(128, KT, JT, 128)
                .transpose(0, 2, 1, 3)
                .reshape(128, KT * FSLICE)
            )
            w12c[:, off + KT * FSLICE : off + 2 * KT * FSLICE] = (
                w2t[s, e]
                .reshape(128, KT, JT, 128)
                .transpose(0, 2, 1, 3)
                .reshape(128, KT * FSLICE)
            )
        w3c = (
            w3t[s][porder].transpose(1, 0, 2, 3).reshape(128, n_experts * JT * D_MODEL)
        )
        in_maps.append(
            {
                "xt": xt,
                "w12": np.ascontiguousarray(w12c),
                "w3p": np.ascontiguousarray(w3c),
            }
        )

    nc = _get_program(caps)
    res = _run(nc, in_maps, trace=_trace)

    # host combine: sum the 8 f-slice partials (fp32), weight, scatter-add
    acc = np.zeros((128, MT * AT), np.float32)
    for s in range(N_CORES):
        acc += np.asarray(res.results[s]["ot"]).astype(np.float32)

    A = len(a_e)
    y = np.empty((A, d_model), np.float32)
    for i, e in enumerate(porder):
        ot = acc[:, MT * offs[i] : MT * (offs[i] + caps[i])]
        ot = (
            ot.reshape(128, MT, caps[i])
            .transpose(1, 0, 2)
            .reshape(d_model, caps[i])
        )
        y[starts[e] : starts[e] + counts[e]] = ot[:, : counts[e]].T
    y *= s_w[:, None]
    y_orig = np.empty_like(y)
    y_orig[order] = y
    out = y_orig[:n_tokens].copy()
    out[~dup] += y_orig[n_tokens:]
    if _trace:
        return out.astype(np.float32, copy=False), res
    return out.astype(np.float32, copy=False)
